# revision 31
# baseline (speedup 1.0000x reference)
"""Cross-attention + FFN + layernorm block on 8 Trainium2 NeuronCores.

Sharding: data-parallel over (B=4) x (LQ split in 2) -> 8 shards of 1024
query rows. Keys/values/weights are replicated per batch; each core runs
the full pipeline for its shard, so no collectives are needed.

Key optimizations over the v1 kernel:
  - The key mask is a prefix mask (positions >= valid_len are masked), so
    fully-masked 128-row key chunks are skipped exactly: KC = ceil(max
    valid_len / 128) chunks instead of LK/128.  Chunks below
    KF = min(floor(valid_len/128)) are fully valid for every batch and
    need no exp bias, which lets two of them share one wide activation.
  - Everything runs in bf16 on the PE (1 cycle/row, fp32 PSUM
    accumulation), with host-side casting.
  - Input transposes (x^T for q/k/v projections) and o^T are done by the
    DMA XBAR transpose engine, not by PE+DVE.
  - att is never materialized row-major: attT = Wo^T @ oT directly, and
    the residual enters the W2 PSUM accumulation as attT^T @ I.  The
    dense2 bias enters as a rank-1 ones x b2 matmul.
  - Attention (ACT-bound on exp) is interleaved with projection and FFN
    matmul chains ("fillers") so the tensor engine never idles and stays
    at its top p-state clock.
  - All Sqrt work (layernorm rstd) is deferred to one tail region so the
    ACT table set is swapped exactly once instead of around every
    layernorm.
"""

import sys

if '/opt/trn_rl_repo' not in sys.path:
    sys.path.insert(0, '/opt/trn_rl_repo')

import numpy as np
import ml_dtypes

B, LQ, LK, D, H = 4, 2048, 2048, 768, 12
DH = D // H            # 64
NC = 8                 # cores
LQC = B * LQ // NC     # 1024 query rows per core
QB = LQC // 128        # 8 q row-tiles
C = D // 128           # 6 feature chunks
EPS = 1e-5

_CACHE = {}


def _build(KC, KF):
    """KC: number of 128-row key chunks kept; KF: chunks < KF are fully
    valid for every batch (no exp bias needed)."""
    import concourse.bacc as bacc
    import concourse.bass as bass
    import concourse.tile as tile
    import concourse.mybir as mybir
    from concourse.masks import make_identity

    f32 = mybir.dt.float32
    bf16 = mybir.dt.bfloat16
    Exp = mybir.ActivationFunctionType.Exp
    Relu = mybir.ActivationFunctionType.Relu
    Sqrt = mybir.ActivationFunctionType.Sqrt
    Copy = mybir.ActivationFunctionType.Copy
    KCT = KC * 128

    nc = bacc.Bacc("TRN2", target_bir_lowering=False, debug=False)

    xq = nc.dram_tensor("xq", [LQC, D], bf16, kind="ExternalInput")
    xk = nc.dram_tensor("xk", [KCT, D], bf16, kind="ExternalInput")
    xv = nc.dram_tensor("xv", [KCT, D], bf16, kind="ExternalInput")
    mbias = nc.dram_tensor("mbias", [128, KC], f32, kind="ExternalInput")
    wq = nc.dram_tensor("wq", [D, D], bf16, kind="ExternalInput")
    wk = nc.dram_tensor("wk", [D, D], bf16, kind="ExternalInput")
    wv = nc.dram_tensor("wv", [D, D], bf16, kind="ExternalInput")
    wo = nc.dram_tensor("wo", [D, D], bf16, kind="ExternalInput")
    w1 = nc.dram_tensor("w1", [D, D], bf16, kind="ExternalInput")
    w2 = nc.dram_tensor("w2", [D, D], bf16, kind="ExternalInput")
    b1c = nc.dram_tensor("b1c", [128, C], f32, kind="ExternalInput")
    b2v = nc.dram_tensor("b2v", [D], bf16, kind="ExternalInput")
    gv = nc.dram_tensor("gv", [D], f32, kind="ExternalInput")
    bv = nc.dram_tensor("bv", [D], f32, kind="ExternalInput")
    yout = nc.dram_tensor("yout", [LQC, D], f32, kind="ExternalOutput")

    def w_ap(w):
        # [128(din part), C(din chunk), D(dout)] view of a [D, D] weight
        return w.ap().rearrange("(c p) n -> p c n", p=128)

    def bcast_ap(v):
        a = v.ap()
        return bass.AP(tensor=a.tensor, offset=a.offset, ap=[[0, 128]] + list(a.ap))

    # exp groups over key chunks: pairs among fully-valid chunks, singles
    # (with mask bias) for the rest
    groups = []
    kc = 0
    while kc + 1 < KF:
        groups.append(((kc, kc + 1), False))
        kc += 2
    if kc < KF:
        groups.append(((kc,), False))
        kc += 1
    while kc < KC:
        groups.append(((kc,), True))
        kc += 1

    with tile.TileContext(nc) as tc:
        with tc.tile_pool(name="consts", bufs=1) as consts, \
             tc.tile_pool(name="persist", bufs=1) as persist, \
             tc.tile_pool(name="work", bufs=2) as work, \
             tc.tile_pool(name="pp", bufs=2, space="PSUM") as pp:

            # ---- constants
            ident = consts.tile([128, 128], bf16)
            make_identity(nc, ident)
            ones_t = consts.tile([128, 128], bf16)
            nc.gpsimd.memset(ones_t[0:1, :], 1.0)
            eps_t = consts.tile([128, 1], f32)
            nc.gpsimd.memset(eps_t, EPS)

            # ---- persistent activations (tags pair disjoint lifetimes)
            qT = persist.tile([128, C, LQC], bf16, tag="sQ")
            kT = persist.tile([128, C, KCT], bf16, tag="sK")
            vp = persist.tile([128, KC, H, DH + 1], bf16, tag="sV")
            xqT = persist.tile([128, C, LQC], bf16, tag="sA")
            xkT = persist.tile([128, C, KCT], bf16, tag="sB")
            xvT = persist.tile([128, C, KCT], bf16, tag="sC")
            o_sb = persist.tile([128, QB, D], bf16, tag="sO")
            hT = persist.tile([128, C, LQC], bf16, tag="sH")
            wq_t = persist.tile([128, C, D], bf16, tag="wA")
            wk_t = persist.tile([128, C, D], bf16, tag="wB")
            wv_t = persist.tile([128, C, D], bf16, tag="wC")

            # denominator ones-column of vp (softmax sum via the o-matmul)
            nc.vector.memset(vp[:, :, :, DH:DH + 1], 1.0)

            # ---- input loads: x^T via DMA XBAR transpose + weights in
            # consumption order.  Issues alternate across the three
            # HWDGE-capable engines (SP/ACT/DVE are all idle at start and
            # a single queue's ~1.2us per-issue rate would gate startup);
            # late-needed broadcast consts (slow gpsimd SWDGE round-trips)
            # go last.
            mb = consts.tile([128, KC], f32)
            nc.gpsimd.dma_start(out=mb, in_=mbias.ap())
            b1_t = consts.tile([128, C], f32)
            nc.gpsimd.dma_start(out=b1_t, in_=b1c.ap())
            b2_t = consts.tile([128, D], bf16)
            nc.gpsimd.dma_start(out=b2_t[0:1, :], in_=b2v.ap())

            for c in range(C):
                nc.sync.dma_start_transpose(
                    xqT[:, c, :], xq.ap()[:, c * 128:(c + 1) * 128])
            nc.gpsimd.dma_start(out=wq_t, in_=w_ap(wq))
            for c in range(C):
                nc.sync.dma_start_transpose(
                    xkT[:, c, :], xk.ap()[:, c * 128:(c + 1) * 128])
            nc.gpsimd.dma_start(out=wk_t, in_=w_ap(wk))
            for c in range(C):
                nc.sync.dma_start_transpose(
                    xvT[:, c, :], xv.ap()[:, c * 128:(c + 1) * 128])
            nc.gpsimd.dma_start(out=wv_t, in_=w_ap(wv))

            # FFN weights prefetched into their own slots
            wo_t = persist.tile([128, C, D], bf16, tag="wD")
            w1_t = persist.tile([128, C, D], bf16, tag="wE")
            w2_t = persist.tile([128, C, D], bf16, tag="wF")
            nc.gpsimd.dma_start(out=wo_t, in_=w_ap(wo))
            nc.gpsimd.dma_start(out=w1_t, in_=w_ap(w1))
            nc.gpsimd.dma_start(out=w2_t, in_=w_ap(w2))
            g_t = consts.tile([128, D], f32)
            nc.gpsimd.dma_start(out=g_t, in_=bcast_ap(gv))
            be_t = consts.tile([128, D], f32)
            nc.gpsimd.dma_start(out=be_t, in_=bcast_ap(bv))
            # oT reuses xkT's slot (xqT stays live: its qt=1 projection
            # chains run as phase-2 fillers after oT writes begin)
            oT = persist.tile([128, C, LQC], bf16, tag="sB")
            attT = persist.tile([128, C, LQC], bf16, tag="sC")

            # eviction engine rotation (keeps DVE from being the choke point)
            ev_state = [0]

            def evict(out_ap, in_ap, engines="sv"):
                e = engines[ev_state[0] % len(engines)]
                ev_state[0] += 1
                if e == "s":
                    nc.scalar.activation(out=out_ap, in_=in_ap, func=Copy,
                                         bias=0.0, scale=1.0)
                else:
                    nc.vector.tensor_copy(out=out_ap, in_=in_ap)

            # ---------------- filler chains ----------------
            def qk_chain(w_t, x_t, out_t, n, q0, qw, engines):
                def fn():
                    ps = pp.tile([128, 512], f32, tag="p512", name=f"ps_{n}_{q0}")
                    for c in range(C):
                        nc.tensor.matmul(ps[:, 0:qw],
                                         w_t[:, c, n * 128:(n + 1) * 128],
                                         x_t[:, c, q0:q0 + qw],
                                         start=(c == 0), stop=(c == C - 1))
                    evict(out_t[:, n, q0:q0 + qw], ps[:, 0:qw], engines)
                return fn

            def v_chain(hp, kt):
                def fn():
                    ps = pp.tile([128, 512], f32, tag="p512", name=f"psv_{hp}_{kt}")
                    for c in range(C):
                        nc.tensor.matmul(ps[:, 0:128],
                                         xvT[:, c, kt * 128:(kt + 1) * 128],
                                         wv_t[:, c, hp * 128:(hp + 1) * 128],
                                         start=(c == 0), stop=(c == C - 1))
                    evict(vp[:, kt, 2 * hp:2 * hp + 2, 0:DH],
                          ps[:, 0:128].rearrange("p (h d) -> p h d", d=DH), "v")
                return fn

            def oT_xbar(c, qbs):
                # 128x128 bf16 SBUF->SBUF transposes on the DMA XBAR
                def fn():
                    for qb in qbs:
                        nc.sync.dma_start_transpose(
                            oT[:, c, qb * 128:(qb + 1) * 128],
                            o_sb[:, qb, c * 128:(c + 1) * 128])
                return fn

            def attT_chain(n, qc):
                # attT[n-chunk, q] = Wo[:, n-chunk]^T @ oT[:, q]
                def fn():
                    ps = pp.tile([128, 512], f32, tag="p512", name=f"psa_{n}_{qc}")
                    for c in range(C):
                        nc.tensor.matmul(ps[:],
                                         wo_t[:, c, n * 128:(n + 1) * 128],
                                         oT[:, c, qc * 512:(qc + 1) * 512],
                                         start=(c == 0), stop=(c == C - 1))
                    evict(attT[:, n, qc * 512:(qc + 1) * 512], ps[:], "v")
                return fn

            def w1_chain(n, qc, engines):
                def fn():
                    ps = pp.tile([128, 512], f32, tag="p512", name=f"ps1_{n}_{qc}")
                    for c in range(C):
                        nc.tensor.matmul(ps[:],
                                         w1_t[:, c, n * 128:(n + 1) * 128],
                                         attT[:, c, qc * 512:(qc + 1) * 512],
                                         start=(c == 0), stop=(c == C - 1))
                    e = engines[ev_state[0] % len(engines)]
                    ev_state[0] += 1
                    if e == "s":
                        nc.scalar.activation(
                            out=hT[:, n, qc * 512:(qc + 1) * 512], in_=ps[:],
                            func=Relu, bias=b1_t[:, n:n + 1], scale=1.0)
                    else:
                        nc.vector.tensor_scalar(
                            out=hT[:, n, qc * 512:(qc + 1) * 512], in0=ps[:],
                            scalar1=b1_t[:, n:n + 1], scalar2=0.0,
                            op0=mybir.AluOpType.add, op1=mybir.AluOpType.max)
                return fn

            # per-qb layernorm state, consumed by the tail region
            ysb_t = [None] * QB
            mv_t = [None] * QB

            def w2_stats_chain(qb):
                # y = hT^T @ W2 + attT^T (residual) + ones x b2, then
                # evict + bn stats; the sqrt-dependent part runs in the tail
                def fn():
                    ps = pp.tile([128, 2, 512], f32, tag="pair", name=f"psy_{qb}")
                    for n0, nw, i in ((0, 512, 0), (512, 256, 1)):
                        for c in range(C):
                            nc.tensor.matmul(ps[:, i, 0:nw],
                                             hT[:, c, qb * 128:(qb + 1) * 128],
                                             w2_t[:, c, n0:n0 + nw],
                                             start=(c == 0), stop=False)
                        # residual: += attT^T via identity rhs, per n-chunk
                        for cn in range(n0 // 128, (n0 + nw) // 128):
                            nc.tensor.matmul(
                                ps[:, i, cn * 128 - n0:(cn + 1) * 128 - n0],
                                attT[:, cn, qb * 128:(qb + 1) * 128],
                                ident[:],
                                start=False, stop=False, skip_group_check=True)
                        # += b2 via rank-1 ones matmul
                        nc.tensor.matmul(ps[:, i, 0:nw], ones_t[0:1, :],
                                         b2_t[0:1, n0:n0 + nw],
                                         start=False, stop=True,
                                         skip_group_check=True)
                    ysb = work.tile([128, D], bf16, tag="ysb", bufs=QB,
                                    name=f"y_{qb}")
                    nc.vector.tensor_copy(out=ysb[:, 0:512], in_=ps[:, 0, :])
                    nc.vector.tensor_copy(out=ysb[:, 512:768],
                                          in_=ps[:, 1, 0:256])
                    stats = work.tile([128, 3, 6], f32, tag="stats",
                                      name=f"st_{qb}")
                    for sg in range(3):
                        nc.vector.bn_stats(out=stats[:, sg, :],
                                           in_=ysb[:, sg * 256:(sg + 1) * 256])
                    mv = work.tile([128, 2], f32, tag="mv", bufs=QB,
                                   name=f"mv_{qb}")
                    nc.vector.bn_aggr(out=mv[:], in_=stats[:])
                    ysb_t[qb] = ysb
                    mv_t[qb] = mv
                return fn

            Identity = mybir.ActivationFunctionType.Identity

            def ln_tail(qb):
                ysb, mv = ysb_t[qb], mv_t[qb]
                rstd = work.tile([128, 1], f32, tag="rstd", bufs=3,
                                 name=f"rs_{qb}")
                nc.scalar.activation(out=rstd[:], in_=mv[:, 1:2], func=Sqrt,
                                     bias=eps_t[:], scale=1.0)
                nc.vector.reciprocal(rstd[:], rstd[:])
                # (y - mu)*rstd on ACT: Identity(y*rstd + (-mu*rstd))
                nmu = work.tile([128, 1], f32, tag="nmu", bufs=3,
                                name=f"nm_{qb}")
                nc.vector.tensor_scalar(
                    out=nmu[:], in0=mv[:, 0:1], scalar1=rstd[:], scalar2=-1.0,
                    op0=mybir.AluOpType.mult, op1=mybir.AluOpType.mult)
                yn = work.tile([128, D], f32, tag="yn", bufs=3, name=f"yn_{qb}")
                nc.scalar.activation(out=yn[:], in_=ysb[:], func=Identity,
                                     bias=nmu[:], scale=rstd[:])
                if qb % 2 == 0:
                    nc.vector.tensor_mul(out=yn[:], in0=yn[:], in1=g_t[:])
                    nc.gpsimd.tensor_add(out=yn[:], in0=yn[:], in1=be_t[:])
                else:
                    nc.gpsimd.tensor_mul(out=yn[:], in0=yn[:], in1=g_t[:])
                    nc.vector.tensor_add(out=yn[:], in0=yn[:], in1=be_t[:])
                nc.scalar.dma_start(out=yout.ap()[qb * 128:(qb + 1) * 128, :],
                                    in_=yn[:])

            # ---------------- filler queue ----------------
            fillers = []   # list of (round, fn)
            fpos = [0]

            def drain(k):
                n = 0
                while fpos[0] < len(fillers) and n < k:
                    fillers[fpos[0]][1]()
                    fpos[0] += 1
                    n += 1

            def force_round(r):
                while fpos[0] < len(fillers) and fillers[fpos[0]][0] <= r:
                    fillers[fpos[0]][1]()
                    fpos[0] += 1

            # phase-1 fillers: qt=0 projections only (qt=1 chains feed the
            # qc=1 attention and become phase-2 fillers for balance)
            kw = [(k0, min(512, KCT - k0)) for k0 in range(0, KCT, 512)]
            for n in range(C):
                fillers.append((n, qk_chain(wq_t, xqT, qT, n, 0, 512, "v")))
                for (k0, wdt) in kw:
                    fillers.append((n, qk_chain(wk_t, xkT, kT, n, k0, wdt,
                                                "v")))
                for kt in range(KC):
                    fillers.append((n, v_chain(n, kt)))

            # rounds 0-1 upfront (attention head 0 needs round 0; round 1
            # keeps PE fed while the head-0 DMAs land)
            force_round(1)

            # ---------------- attention ----------------
            # software-pipelined: scores for step idx+1 are emitted before
            # the o-matmuls of step idx, so the PE never sits behind the
            # exp latency
            NG = len(groups)

            def attn_phase(qc, post_head=None):
                seq = [(h, gi) for h in range(H) for gi in range(NG)]
                pos_t = {}
                pair_t = {}

                def emit_scores(idx):
                    h, gi = seq[idx]
                    p0 = (h % 2) * 64
                    cc = h // 2
                    if gi == 0:
                        force_round(min(cc + 2, C - 1))
                        pos = pp.tile([128, 4, DH + 1], f32, tag="pos",
                                      name=f"pos_{qc}_{h}")
                        # 4 accumulation groups share this bank: a
                        # start=True reset wipes the whole bank, so zero it
                        # once and accumulate only
                        nc.vector.memset(pos[:], 0.0)
                        pos_t[h] = pos
                    g, _ = groups[gi]
                    ps_s = pp.tile([128, 2, 512], f32, tag="pair",
                                   name=f"pss_{qc}_{h}_{gi}")
                    for i, kc in enumerate(g):
                        nc.tensor.matmul(
                            ps_s[:, i, :],
                            kT[p0:p0 + 64, cc, kc * 128:(kc + 1) * 128],
                            qT[p0:p0 + 64, cc, qc * 512:(qc + 1) * 512],
                            start=True, stop=True)
                    pair_t[(h, gi)] = ps_s

                emit_scores(0)
                for idx, (h, gi) in enumerate(seq):
                    g, biased = groups[gi]
                    ps_s = pair_t.pop((h, gi))
                    ex = work.tile([128, 2, 512], bf16, tag="ex", bufs=3,
                                   name=f"ex_{qc}_{h}_{gi}")
                    glen = len(g)
                    bias = mb[:, g[0]:g[0] + 1] if biased else 0.0
                    nc.scalar.activation(out=ex[:, 0:glen, :],
                                         in_=ps_s[:, 0:glen, :], func=Exp,
                                         bias=bias, scale=1.0)
                    if idx + 1 < len(seq):
                        emit_scores(idx + 1)
                    drain(1)
                    pos = pos_t[h]
                    for i, kc in enumerate(g):
                        for qs in range(4):
                            nc.tensor.matmul(
                                pos[:, qs, :],
                                ex[:, i, qs * 128:(qs + 1) * 128],
                                vp[:, kc, h, :],
                                start=False, stop=(kc == KC - 1),
                                skip_group_check=True)
                    if gi == NG - 1:
                        # normalize -> o_sb
                        pos = pos_t.pop(h)
                        rec = work.tile([128, 4, 1], f32, tag="rec", bufs=3,
                                        name=f"rec_{qc}_{h}")
                        nc.vector.reciprocal(rec[:], pos[:, :, DH:DH + 1])
                        for qs in range(4):
                            qb = qc * 4 + qs
                            nc.vector.tensor_scalar_mul(
                                out=o_sb[:, qb, h * DH:(h + 1) * DH],
                                in0=pos[:, qs, 0:DH],
                                scalar1=rec[:, qs, 0:1])
                        if post_head is not None:
                            post_head(h)

            # qc=0: fillers are the projection chains
            attn_phase(0)
            force_round(C)

            # qc=1: fillers are the qt=1 projections + the qc=0 FFN pipeline
            fillers.clear()
            fpos[0] = 0
            for c in range(C):
                fillers.append((0, oT_xbar(c, (0, 1))))
                fillers.append((0, oT_xbar(c, (2, 3))))
            for n in range(C):
                fillers.append((n, qk_chain(wq_t, xqT, qT, n, 512, 512, "v")))
                fillers.append((n, attT_chain(n, 0)))
            for n in range(C):
                # w1 contracts over ALL attT chunks -> must follow attT(5)
                fillers.append((C - 1, w1_chain(n, 0, "v")))
            for qb in range(4):
                fillers.append((C, w2_stats_chain(qb)))

            def post_head_qc1(h):
                if h % 2 == 1:
                    # o_sb columns for chunk h//2 are complete for qc=1
                    c = h // 2
                    fillers.append((C, oT_xbar(c, (4, 5))))
                    fillers.append((C, oT_xbar(c, (6, 7))))

            attn_phase(1, post_head=post_head_qc1)
            drain(len(fillers))

            # ---------------- qc=1 FFN tail ----------------
            for n in range(C):
                attT_chain(n, 1)()
            for n in range(C):
                w1_chain(n, 1, "sv")()
            for qb in range(4, QB):
                w2_stats_chain(qb)()
            # layernorm tail: all Sqrt usage lives here, so the ACT table
            # set is swapped exactly once for the whole kernel
            for qb in range(QB):
                ln_tail(qb)

    nc.compile()
    return nc


def _get_nc():
    # returns the most recently used compiled module (for test harness)
    key = _CACHE.get("last_key")
    if key is None:
        # default shape for this problem's input (valid_lens ~1028/1044/996)
        key = (9, 7)
    if ("nc", key) not in _CACHE:
        _CACHE[("nc", key)] = _build(*key)
    _CACHE["last_key"] = key
    return _CACHE[("nc", key)]


def _prepare_in_maps(queries, keys, values, mask, Wq, Wk, Wv, Wo, W1, b1,
                     W2, b2, ln_g, ln_b):
    bf16 = ml_dtypes.bfloat16
    queries = np.asarray(queries, dtype=np.float32)
    keys = np.asarray(keys, dtype=np.float32)
    values = np.asarray(values, dtype=np.float32)
    mask = np.asarray(mask)

    valid = (mask != 0).sum(axis=1).astype(np.int64)        # [B]
    valid = np.maximum(valid, 1)
    KC = int(-(-valid.max() // 128))
    KF = int(valid.min() // 128)
    KCT = KC * 128

    kidx = np.arange(KCT)
    mb_all = np.where(kidx[None, :] < valid[:, None], 0.0, -1e6)
    mb_all = mb_all.astype(np.float32).reshape(B, KC, 128).transpose(0, 2, 1)

    wq_s = (np.asarray(Wq, np.float32) / np.sqrt(np.float32(DH)))
    common = {
        "wq": wq_s.astype(bf16),
        "wk": np.asarray(Wk, np.float32).astype(bf16),
        "wv": np.asarray(Wv, np.float32).astype(bf16),
        "wo": np.asarray(Wo, np.float32).astype(bf16),
        "w1": np.asarray(W1, np.float32).astype(bf16),
        "w2": np.asarray(W2, np.float32).astype(bf16),
        "b1c": np.ascontiguousarray(
            np.asarray(b1, np.float32).reshape(C, 128).T),
        "b2v": np.asarray(b2, np.float32).astype(bf16),
        "gv": np.ascontiguousarray(ln_g, np.float32),
        "bv": np.ascontiguousarray(ln_b, np.float32),
    }

    in_maps = []
    for core in range(NC):
        b, half = core // 2, core % 2
        in_maps.append(dict(
            common,
            xq=np.ascontiguousarray(
                queries[b, half * LQC:(half + 1) * LQC, :]).astype(bf16),
            xk=np.ascontiguousarray(keys[b, :KCT, :]).astype(bf16),
            xv=np.ascontiguousarray(values[b, :KCT, :]).astype(bf16),
            mbias=np.ascontiguousarray(mb_all[b]),
        ))
    return in_maps, (KC, KF)


def kernel(queries, keys, values, mask, Wq, Wk, Wv, Wo, W1, b1, W2, b2,
           ln_g, ln_b, _trace=False):
    from concourse.bass_utils import run_bass_kernel_spmd

    in_maps, key = _prepare_in_maps(queries, keys, values, mask, Wq, Wk, Wv,
                                    Wo, W1, b1, W2, b2, ln_g, ln_b)
    if ("nc", key) not in _CACHE:
        _CACHE[("nc", key)] = _build(*key)
    _CACHE["last_key"] = key
    nc = _CACHE[("nc", key)]
    res = run_bass_kernel_spmd(nc, in_maps, core_ids=list(range(NC)),
                               trace=_trace)
    _CACHE["last_result"] = res

    out = np.empty((B, LQ, D), dtype=np.float32)
    for core in range(NC):
        b, half = core // 2, core % 2
        out[b, half * LQC:(half + 1) * LQC, :] = res.results[core]["yout"]
    return out


# revision 32
# speedup vs baseline: 1.0918x; 1.0918x over previous
"""Cross-attention + FFN + layernorm block on 8 Trainium2 NeuronCores.

Sharding: data-parallel over (B=4) x (LQ split in 2) -> 8 shards of 1024
query rows. Keys/values/weights are replicated per batch; each core runs
the full pipeline for its shard, so no collectives are needed.

Key optimizations over the v1 kernel:
  - The key mask is a prefix mask (positions >= valid_len are masked), so
    fully-masked 128-row key chunks are skipped exactly: KC = ceil(max
    valid_len / 128) chunks instead of LK/128.  Chunks below
    KF = min(floor(valid_len/128)) are fully valid for every batch and
    need no exp bias, which lets two of them share one wide activation.
  - Everything runs in bf16 on the PE (1 cycle/row, fp32 PSUM
    accumulation), with host-side casting.
  - Input transposes (x^T for q/k/v projections) and o^T are done by the
    DMA XBAR transpose engine, not by PE+DVE.
  - att is never materialized row-major: attT = Wo^T @ oT directly, and
    the residual enters the W2 PSUM accumulation as attT^T @ I.  The
    dense2 bias enters as a rank-1 ones x b2 matmul.
  - Attention (ACT-bound on exp) is interleaved with projection and FFN
    matmul chains ("fillers") so the tensor engine never idles and stays
    at its top p-state clock.
  - All Sqrt work (layernorm rstd) is deferred to one tail region so the
    ACT table set is swapped exactly once instead of around every
    layernorm.
"""

import sys

if '/opt/trn_rl_repo' not in sys.path:
    sys.path.insert(0, '/opt/trn_rl_repo')

import numpy as np
import ml_dtypes

B, LQ, LK, D, H = 4, 2048, 2048, 768, 12
DH = D // H            # 64
NC = 8                 # cores
LQC = B * LQ // NC     # 1024 query rows per core
QB = LQC // 128        # 8 q row-tiles
C = D // 128           # 6 feature chunks
EPS = 1e-5

_CACHE = {}


def _build(KC, KF):
    """KC: number of 128-row key chunks kept; KF: chunks < KF are fully
    valid for every batch (no exp bias needed)."""
    import concourse.bacc as bacc
    import concourse.bass as bass
    import concourse.tile as tile
    import concourse.mybir as mybir
    from concourse.masks import make_identity

    f32 = mybir.dt.float32
    bf16 = mybir.dt.bfloat16
    Exp = mybir.ActivationFunctionType.Exp
    Relu = mybir.ActivationFunctionType.Relu
    Sqrt = mybir.ActivationFunctionType.Sqrt
    Copy = mybir.ActivationFunctionType.Copy
    KCT = KC * 128

    nc = bacc.Bacc("TRN2", target_bir_lowering=False, debug=False)

    xq = nc.dram_tensor("xq", [LQC, D], bf16, kind="ExternalInput")
    xk = nc.dram_tensor("xk", [KCT, D], bf16, kind="ExternalInput")
    xv = nc.dram_tensor("xv", [KCT, D], bf16, kind="ExternalInput")
    mbias = nc.dram_tensor("mbias", [128, KC], f32, kind="ExternalInput")
    wq = nc.dram_tensor("wq", [D, D], bf16, kind="ExternalInput")
    wk = nc.dram_tensor("wk", [D, D], bf16, kind="ExternalInput")
    wv = nc.dram_tensor("wv", [D, D], bf16, kind="ExternalInput")
    wo = nc.dram_tensor("wo", [D, D], bf16, kind="ExternalInput")
    w1 = nc.dram_tensor("w1", [D, D], bf16, kind="ExternalInput")
    w2 = nc.dram_tensor("w2", [D, D], bf16, kind="ExternalInput")
    b1c = nc.dram_tensor("b1c", [128, C], f32, kind="ExternalInput")
    b2v = nc.dram_tensor("b2v", [D], bf16, kind="ExternalInput")
    gv = nc.dram_tensor("gv", [D], f32, kind="ExternalInput")
    bv = nc.dram_tensor("bv", [D], f32, kind="ExternalInput")
    yout = nc.dram_tensor("yout", [LQC, D], f32, kind="ExternalOutput")

    def w_ap(w):
        # [128(din part), C(din chunk), D(dout)] view of a [D, D] weight
        return w.ap().rearrange("(c p) n -> p c n", p=128)

    def bcast_ap(v):
        a = v.ap()
        return bass.AP(tensor=a.tensor, offset=a.offset, ap=[[0, 128]] + list(a.ap))

    # exp groups over key chunks: pairs among fully-valid chunks, singles
    # (with mask bias) for the rest
    groups = []
    kc = 0
    while kc + 1 < KF:
        groups.append(((kc, kc + 1), False))
        kc += 2
    if kc < KF:
        groups.append(((kc,), False))
        kc += 1
    while kc < KC:
        groups.append(((kc,), True))
        kc += 1

    with tile.TileContext(nc) as tc:
        with tc.tile_pool(name="consts", bufs=1) as consts, \
             tc.tile_pool(name="persist", bufs=1) as persist, \
             tc.tile_pool(name="work", bufs=2) as work, \
             tc.tile_pool(name="pp", bufs=2, space="PSUM") as pp:

            # ---- constants
            ident = consts.tile([128, 128], bf16)
            make_identity(nc, ident)
            ones_t = consts.tile([128, 128], bf16)
            nc.gpsimd.memset(ones_t[0:1, :], 1.0)
            eps_t = consts.tile([128, 1], f32)
            nc.gpsimd.memset(eps_t, EPS)

            # ---- persistent activations (tags pair disjoint lifetimes)
            qT = persist.tile([128, C, LQC], bf16, tag="sQ")
            kT = persist.tile([128, C, KCT], bf16, tag="sK")
            vp = persist.tile([128, KC, H, DH + 1], bf16, tag="sV")
            xqT = persist.tile([128, C, LQC], bf16, tag="sA")
            xkT = persist.tile([128, C, KCT], bf16, tag="sB")
            xvT = persist.tile([128, C, KCT], bf16, tag="sC")
            o_sb = persist.tile([128, QB, D], bf16, tag="sO")
            hT = persist.tile([128, C, LQC], bf16, tag="sH")
            wq_t = persist.tile([128, C, D], bf16, tag="wA")
            wk_t = persist.tile([128, C, D], bf16, tag="wB")
            wv_t = persist.tile([128, C, D], bf16, tag="wC")

            # denominator ones-column of vp (softmax sum via the o-matmul)
            nc.vector.memset(vp[:, :, :, DH:DH + 1], 1.0)

            # ---- input loads: x^T via DMA XBAR transpose + weights in
            # consumption order.  Issues alternate across the three
            # HWDGE-capable engines (SP/ACT/DVE are all idle at start and
            # a single queue's ~1.2us per-issue rate would gate startup);
            # late-needed broadcast consts (slow gpsimd SWDGE round-trips)
            # go last.
            mb = consts.tile([128, KC], f32)
            nc.sync.dma_start(out=mb, in_=mbias.ap())
            b1_t = consts.tile([128, C], f32)
            nc.sync.dma_start(out=b1_t, in_=b1c.ap())
            b2_t = consts.tile([128, D], bf16)
            nc.sync.dma_start(out=b2_t[0:1, :], in_=b2v.ap())

            for c in range(C):
                nc.sync.dma_start_transpose(
                    xqT[:, c, :], xq.ap()[:, c * 128:(c + 1) * 128])
            nc.sync.dma_start(out=wq_t, in_=w_ap(wq))
            for c in range(C):
                nc.sync.dma_start_transpose(
                    xkT[:, c, :], xk.ap()[:, c * 128:(c + 1) * 128])
            nc.sync.dma_start(out=wk_t, in_=w_ap(wk))
            for c in range(C):
                nc.sync.dma_start_transpose(
                    xvT[:, c, :], xv.ap()[:, c * 128:(c + 1) * 128])
            nc.sync.dma_start(out=wv_t, in_=w_ap(wv))

            # FFN weights prefetched into their own slots
            wo_t = persist.tile([128, C, D], bf16, tag="wD")
            w1_t = persist.tile([128, C, D], bf16, tag="wE")
            w2_t = persist.tile([128, C, D], bf16, tag="wF")
            nc.sync.dma_start(out=wo_t, in_=w_ap(wo))
            nc.sync.dma_start(out=w1_t, in_=w_ap(w1))
            nc.sync.dma_start(out=w2_t, in_=w_ap(w2))
            g_t = consts.tile([128, D], f32)
            nc.gpsimd.dma_start(out=g_t, in_=bcast_ap(gv))
            be_t = consts.tile([128, D], f32)
            nc.gpsimd.dma_start(out=be_t, in_=bcast_ap(bv))
            # oT reuses xkT's slot (xqT stays live: its qt=1 projection
            # chains run as phase-2 fillers after oT writes begin)
            oT = persist.tile([128, C, LQC], bf16, tag="sB")
            attT = persist.tile([128, C, LQC], bf16, tag="sC")

            # eviction engine rotation (keeps DVE from being the choke point)
            ev_state = [0]

            def evict(out_ap, in_ap, engines="sv"):
                e = engines[ev_state[0] % len(engines)]
                ev_state[0] += 1
                if e == "s":
                    nc.scalar.activation(out=out_ap, in_=in_ap, func=Copy,
                                         bias=0.0, scale=1.0)
                else:
                    nc.vector.tensor_copy(out=out_ap, in_=in_ap)

            # ---------------- filler chains ----------------
            def qk_chain(w_t, x_t, out_t, n, q0, qw, engines):
                def fn():
                    ps = pp.tile([128, 512], f32, tag="p512", name=f"ps_{n}_{q0}")
                    for c in range(C):
                        nc.tensor.matmul(ps[:, 0:qw],
                                         w_t[:, c, n * 128:(n + 1) * 128],
                                         x_t[:, c, q0:q0 + qw],
                                         start=(c == 0), stop=(c == C - 1))
                    evict(out_t[:, n, q0:q0 + qw], ps[:, 0:qw], engines)
                return fn

            def v_chain(hp, kt):
                def fn():
                    ps = pp.tile([128, 512], f32, tag="p512", name=f"psv_{hp}_{kt}")
                    for c in range(C):
                        nc.tensor.matmul(ps[:, 0:128],
                                         xvT[:, c, kt * 128:(kt + 1) * 128],
                                         wv_t[:, c, hp * 128:(hp + 1) * 128],
                                         start=(c == 0), stop=(c == C - 1))
                    evict(vp[:, kt, 2 * hp:2 * hp + 2, 0:DH],
                          ps[:, 0:128].rearrange("p (h d) -> p h d", d=DH), "v")
                return fn

            def oT_xbar(c, qbs):
                # 128x128 bf16 SBUF->SBUF transposes on the DMA XBAR
                def fn():
                    for qb in qbs:
                        nc.sync.dma_start_transpose(
                            oT[:, c, qb * 128:(qb + 1) * 128],
                            o_sb[:, qb, c * 128:(c + 1) * 128])
                return fn

            def attT_chain(n, qc):
                # attT[n-chunk, q] = Wo[:, n-chunk]^T @ oT[:, q]
                def fn():
                    ps = pp.tile([128, 512], f32, tag="p512", name=f"psa_{n}_{qc}")
                    for c in range(C):
                        nc.tensor.matmul(ps[:],
                                         wo_t[:, c, n * 128:(n + 1) * 128],
                                         oT[:, c, qc * 512:(qc + 1) * 512],
                                         start=(c == 0), stop=(c == C - 1))
                    evict(attT[:, n, qc * 512:(qc + 1) * 512], ps[:], "v")
                return fn

            def w1_chain(n, qc, engines):
                def fn():
                    ps = pp.tile([128, 512], f32, tag="p512", name=f"ps1_{n}_{qc}")
                    for c in range(C):
                        nc.tensor.matmul(ps[:],
                                         w1_t[:, c, n * 128:(n + 1) * 128],
                                         attT[:, c, qc * 512:(qc + 1) * 512],
                                         start=(c == 0), stop=(c == C - 1))
                    e = engines[ev_state[0] % len(engines)]
                    ev_state[0] += 1
                    if e == "s":
                        nc.scalar.activation(
                            out=hT[:, n, qc * 512:(qc + 1) * 512], in_=ps[:],
                            func=Relu, bias=b1_t[:, n:n + 1], scale=1.0)
                    else:
                        nc.vector.tensor_scalar(
                            out=hT[:, n, qc * 512:(qc + 1) * 512], in0=ps[:],
                            scalar1=b1_t[:, n:n + 1], scalar2=0.0,
                            op0=mybir.AluOpType.add, op1=mybir.AluOpType.max)
                return fn

            # per-qb layernorm state, consumed by the tail region
            ysb_t = [None] * QB
            mv_t = [None] * QB

            def w2_stats_chain(qb):
                # y = hT^T @ W2 + attT^T (residual) + ones x b2, then
                # evict + bn stats; the sqrt-dependent part runs in the tail
                def fn():
                    ps = pp.tile([128, 2, 512], f32, tag="pair", name=f"psy_{qb}")
                    for n0, nw, i in ((0, 512, 0), (512, 256, 1)):
                        for c in range(C):
                            nc.tensor.matmul(ps[:, i, 0:nw],
                                             hT[:, c, qb * 128:(qb + 1) * 128],
                                             w2_t[:, c, n0:n0 + nw],
                                             start=(c == 0), stop=False)
                        # residual: += attT^T via identity rhs, per n-chunk
                        for cn in range(n0 // 128, (n0 + nw) // 128):
                            nc.tensor.matmul(
                                ps[:, i, cn * 128 - n0:(cn + 1) * 128 - n0],
                                attT[:, cn, qb * 128:(qb + 1) * 128],
                                ident[:],
                                start=False, stop=False, skip_group_check=True)
                        # += b2 via rank-1 ones matmul
                        nc.tensor.matmul(ps[:, i, 0:nw], ones_t[0:1, :],
                                         b2_t[0:1, n0:n0 + nw],
                                         start=False, stop=True,
                                         skip_group_check=True)
                    ysb = work.tile([128, D], bf16, tag="ysb", bufs=QB,
                                    name=f"y_{qb}")
                    nc.vector.tensor_copy(out=ysb[:, 0:512], in_=ps[:, 0, :])
                    nc.vector.tensor_copy(out=ysb[:, 512:768],
                                          in_=ps[:, 1, 0:256])
                    stats = work.tile([128, 3, 6], f32, tag="stats",
                                      name=f"st_{qb}")
                    for sg in range(3):
                        nc.vector.bn_stats(out=stats[:, sg, :],
                                           in_=ysb[:, sg * 256:(sg + 1) * 256])
                    mv = work.tile([128, 2], f32, tag="mv", bufs=QB,
                                   name=f"mv_{qb}")
                    nc.vector.bn_aggr(out=mv[:], in_=stats[:])
                    ysb_t[qb] = ysb
                    mv_t[qb] = mv
                return fn

            Identity = mybir.ActivationFunctionType.Identity

            def ln_tail(qb):
                ysb, mv = ysb_t[qb], mv_t[qb]
                rstd = work.tile([128, 1], f32, tag="rstd", bufs=3,
                                 name=f"rs_{qb}")
                nc.scalar.activation(out=rstd[:], in_=mv[:, 1:2], func=Sqrt,
                                     bias=eps_t[:], scale=1.0)
                nc.vector.reciprocal(rstd[:], rstd[:])
                # (y - mu)*rstd on ACT: Identity(y*rstd + (-mu*rstd))
                nmu = work.tile([128, 1], f32, tag="nmu", bufs=3,
                                name=f"nm_{qb}")
                nc.vector.tensor_scalar(
                    out=nmu[:], in0=mv[:, 0:1], scalar1=rstd[:], scalar2=-1.0,
                    op0=mybir.AluOpType.mult, op1=mybir.AluOpType.mult)
                yn = work.tile([128, D], f32, tag="yn", bufs=3, name=f"yn_{qb}")
                nc.scalar.activation(out=yn[:], in_=ysb[:], func=Identity,
                                     bias=nmu[:], scale=rstd[:])
                if qb % 2 == 0:
                    nc.vector.tensor_mul(out=yn[:], in0=yn[:], in1=g_t[:])
                    nc.gpsimd.tensor_add(out=yn[:], in0=yn[:], in1=be_t[:])
                else:
                    nc.gpsimd.tensor_mul(out=yn[:], in0=yn[:], in1=g_t[:])
                    nc.vector.tensor_add(out=yn[:], in0=yn[:], in1=be_t[:])
                nc.scalar.dma_start(out=yout.ap()[qb * 128:(qb + 1) * 128, :],
                                    in_=yn[:])

            # ---------------- filler queue ----------------
            fillers = []   # list of (round, fn)
            fpos = [0]

            def drain(k):
                n = 0
                while fpos[0] < len(fillers) and n < k:
                    fillers[fpos[0]][1]()
                    fpos[0] += 1
                    n += 1

            def force_round(r):
                while fpos[0] < len(fillers) and fillers[fpos[0]][0] <= r:
                    fillers[fpos[0]][1]()
                    fpos[0] += 1

            # phase-1 fillers: qt=0 projections only (qt=1 chains feed the
            # qc=1 attention and become phase-2 fillers for balance)
            kw = [(k0, min(512, KCT - k0)) for k0 in range(0, KCT, 512)]
            for n in range(C):
                fillers.append((n, qk_chain(wq_t, xqT, qT, n, 0, 512, "v")))
                for (k0, wdt) in kw:
                    fillers.append((n, qk_chain(wk_t, xkT, kT, n, k0, wdt,
                                                "v")))
                for kt in range(KC):
                    fillers.append((n, v_chain(n, kt)))

            # rounds 0-1 upfront (attention head 0 needs round 0; round 1
            # keeps PE fed while the head-0 DMAs land)
            force_round(1)

            # ---------------- attention ----------------
            # software-pipelined: scores for step idx+1 are emitted before
            # the o-matmuls of step idx, so the PE never sits behind the
            # exp latency
            NG = len(groups)

            def attn_phase(qc, post_head=None):
                seq = [(h, gi) for h in range(H) for gi in range(NG)]
                pos_t = {}
                pair_t = {}

                def emit_scores(idx):
                    h, gi = seq[idx]
                    p0 = (h % 2) * 64
                    cc = h // 2
                    if gi == 0:
                        force_round(min(cc + 2, C - 1))
                        pos = pp.tile([128, 4, DH + 1], f32, tag="pos",
                                      name=f"pos_{qc}_{h}")
                        # 4 accumulation groups share this bank: a
                        # start=True reset wipes the whole bank, so zero it
                        # once and accumulate only
                        nc.vector.memset(pos[:], 0.0)
                        pos_t[h] = pos
                    g, _ = groups[gi]
                    ps_s = pp.tile([128, 2, 512], f32, tag="pair",
                                   name=f"pss_{qc}_{h}_{gi}")
                    for i, kc in enumerate(g):
                        nc.tensor.matmul(
                            ps_s[:, i, :],
                            kT[p0:p0 + 64, cc, kc * 128:(kc + 1) * 128],
                            qT[p0:p0 + 64, cc, qc * 512:(qc + 1) * 512],
                            start=True, stop=True)
                    pair_t[(h, gi)] = ps_s

                emit_scores(0)
                for idx, (h, gi) in enumerate(seq):
                    g, biased = groups[gi]
                    ps_s = pair_t.pop((h, gi))
                    ex = work.tile([128, 2, 512], bf16, tag="ex", bufs=3,
                                   name=f"ex_{qc}_{h}_{gi}")
                    glen = len(g)
                    bias = mb[:, g[0]:g[0] + 1] if biased else 0.0
                    nc.scalar.activation(out=ex[:, 0:glen, :],
                                         in_=ps_s[:, 0:glen, :], func=Exp,
                                         bias=bias, scale=1.0)
                    if idx + 1 < len(seq):
                        emit_scores(idx + 1)
                    drain(1)
                    pos = pos_t[h]
                    for i, kc in enumerate(g):
                        for qs in range(4):
                            nc.tensor.matmul(
                                pos[:, qs, :],
                                ex[:, i, qs * 128:(qs + 1) * 128],
                                vp[:, kc, h, :],
                                start=False, stop=(kc == KC - 1),
                                skip_group_check=True)
                    if gi == NG - 1:
                        # normalize -> o_sb
                        pos = pos_t.pop(h)
                        rec = work.tile([128, 4, 1], f32, tag="rec", bufs=3,
                                        name=f"rec_{qc}_{h}")
                        nc.vector.reciprocal(rec[:], pos[:, :, DH:DH + 1])
                        for qs in range(4):
                            qb = qc * 4 + qs
                            nc.vector.tensor_scalar_mul(
                                out=o_sb[:, qb, h * DH:(h + 1) * DH],
                                in0=pos[:, qs, 0:DH],
                                scalar1=rec[:, qs, 0:1])
                        if post_head is not None:
                            post_head(h)

            # qc=0: fillers are the projection chains
            attn_phase(0)
            force_round(C)

            # qc=1: fillers are the qt=1 projections + the qc=0 FFN pipeline
            fillers.clear()
            fpos[0] = 0
            for c in range(C):
                fillers.append((0, oT_xbar(c, (0, 1))))
                fillers.append((0, oT_xbar(c, (2, 3))))
            for n in range(C):
                fillers.append((n, qk_chain(wq_t, xqT, qT, n, 512, 512, "v")))
                fillers.append((n, attT_chain(n, 0)))
            for n in range(C):
                # w1 contracts over ALL attT chunks -> must follow attT(5)
                fillers.append((C - 1, w1_chain(n, 0, "v")))
            for qb in range(4):
                fillers.append((C, w2_stats_chain(qb)))

            def post_head_qc1(h):
                if h % 2 == 1:
                    # o_sb columns for chunk h//2 are complete for qc=1
                    c = h // 2
                    fillers.append((C, oT_xbar(c, (4, 5))))
                    fillers.append((C, oT_xbar(c, (6, 7))))

            attn_phase(1, post_head=post_head_qc1)
            drain(len(fillers))

            # ---------------- qc=1 FFN tail ----------------
            for n in range(C):
                attT_chain(n, 1)()
            for n in range(C):
                w1_chain(n, 1, "sv")()
            for qb in range(4, QB):
                w2_stats_chain(qb)()
            # layernorm tail: all Sqrt usage lives here, so the ACT table
            # set is swapped exactly once for the whole kernel
            for qb in range(QB):
                ln_tail(qb)

    nc.compile()
    return nc


def _get_nc():
    # returns the most recently used compiled module (for test harness)
    key = _CACHE.get("last_key")
    if key is None:
        # default shape for this problem's input (valid_lens ~1028/1044/996)
        key = (9, 7)
    if ("nc", key) not in _CACHE:
        _CACHE[("nc", key)] = _build(*key)
    _CACHE["last_key"] = key
    return _CACHE[("nc", key)]


def _prepare_in_maps(queries, keys, values, mask, Wq, Wk, Wv, Wo, W1, b1,
                     W2, b2, ln_g, ln_b):
    bf16 = ml_dtypes.bfloat16
    queries = np.asarray(queries, dtype=np.float32)
    keys = np.asarray(keys, dtype=np.float32)
    values = np.asarray(values, dtype=np.float32)
    mask = np.asarray(mask)

    valid = (mask != 0).sum(axis=1).astype(np.int64)        # [B]
    valid = np.maximum(valid, 1)
    KC = int(-(-valid.max() // 128))
    KF = int(valid.min() // 128)
    KCT = KC * 128

    kidx = np.arange(KCT)
    mb_all = np.where(kidx[None, :] < valid[:, None], 0.0, -1e6)
    mb_all = mb_all.astype(np.float32).reshape(B, KC, 128).transpose(0, 2, 1)

    wq_s = (np.asarray(Wq, np.float32) / np.sqrt(np.float32(DH)))
    common = {
        "wq": wq_s.astype(bf16),
        "wk": np.asarray(Wk, np.float32).astype(bf16),
        "wv": np.asarray(Wv, np.float32).astype(bf16),
        "wo": np.asarray(Wo, np.float32).astype(bf16),
        "w1": np.asarray(W1, np.float32).astype(bf16),
        "w2": np.asarray(W2, np.float32).astype(bf16),
        "b1c": np.ascontiguousarray(
            np.asarray(b1, np.float32).reshape(C, 128).T),
        "b2v": np.asarray(b2, np.float32).astype(bf16),
        "gv": np.ascontiguousarray(ln_g, np.float32),
        "bv": np.ascontiguousarray(ln_b, np.float32),
    }

    in_maps = []
    for core in range(NC):
        b, half = core // 2, core % 2
        in_maps.append(dict(
            common,
            xq=np.ascontiguousarray(
                queries[b, half * LQC:(half + 1) * LQC, :]).astype(bf16),
            xk=np.ascontiguousarray(keys[b, :KCT, :]).astype(bf16),
            xv=np.ascontiguousarray(values[b, :KCT, :]).astype(bf16),
            mbias=np.ascontiguousarray(mb_all[b]),
        ))
    return in_maps, (KC, KF)


def kernel(queries, keys, values, mask, Wq, Wk, Wv, Wo, W1, b1, W2, b2,
           ln_g, ln_b, _trace=False):
    from concourse.bass_utils import run_bass_kernel_spmd

    in_maps, key = _prepare_in_maps(queries, keys, values, mask, Wq, Wk, Wv,
                                    Wo, W1, b1, W2, b2, ln_g, ln_b)
    if ("nc", key) not in _CACHE:
        _CACHE[("nc", key)] = _build(*key)
    _CACHE["last_key"] = key
    nc = _CACHE[("nc", key)]
    res = run_bass_kernel_spmd(nc, in_maps, core_ids=list(range(NC)),
                               trace=_trace)
    _CACHE["last_result"] = res

    out = np.empty((B, LQ, D), dtype=np.float32)
    for core in range(NC):
        b, half = core // 2, core % 2
        out[b, half * LQC:(half + 1) * LQC, :] = res.results[core]["yout"]
    return out


# revision 33
# speedup vs baseline: 1.1209x; 1.0266x over previous
"""Cross-attention + FFN + layernorm block on 8 Trainium2 NeuronCores.

Sharding: data-parallel over (B=4) x (LQ split in 2) -> 8 shards of 1024
query rows. Keys/values/weights are replicated per batch; each core runs
the full pipeline for its shard, so no collectives are needed.

Key optimizations over the v1 kernel:
  - The key mask is a prefix mask (positions >= valid_len are masked), so
    fully-masked 128-row key chunks are skipped exactly: KC = ceil(max
    valid_len / 128) chunks instead of LK/128.  Chunks below
    KF = min(floor(valid_len/128)) are fully valid for every batch and
    need no exp bias, which lets two of them share one wide activation.
  - Everything runs in bf16 on the PE (1 cycle/row, fp32 PSUM
    accumulation), with host-side casting.
  - Input transposes (x^T for q/k/v projections) and o^T are done by the
    DMA XBAR transpose engine, not by PE+DVE.
  - att is never materialized row-major: attT = Wo^T @ oT directly, and
    the residual enters the W2 PSUM accumulation as attT^T @ I.  The
    dense2 bias enters as a rank-1 ones x b2 matmul.
  - Attention (ACT-bound on exp) is interleaved with projection and FFN
    matmul chains ("fillers") so the tensor engine never idles and stays
    at its top p-state clock.
  - All Sqrt work (layernorm rstd) is deferred to one tail region so the
    ACT table set is swapped exactly once instead of around every
    layernorm.
"""

import sys

if '/opt/trn_rl_repo' not in sys.path:
    sys.path.insert(0, '/opt/trn_rl_repo')

import numpy as np
import ml_dtypes

B, LQ, LK, D, H = 4, 2048, 2048, 768, 12
DH = D // H            # 64
NC = 8                 # cores
LQC = B * LQ // NC     # 1024 query rows per core
QB = LQC // 128        # 8 q row-tiles
C = D // 128           # 6 feature chunks
EPS = 1e-5

_CACHE = {}


def _build(KC, KF):
    """KC: number of 128-row key chunks kept; KF: chunks < KF are fully
    valid for every batch (no exp bias needed)."""
    import concourse.bacc as bacc
    import concourse.bass as bass
    import concourse.tile as tile
    import concourse.mybir as mybir
    from concourse.masks import make_identity

    f32 = mybir.dt.float32
    bf16 = mybir.dt.bfloat16
    Exp = mybir.ActivationFunctionType.Exp
    Relu = mybir.ActivationFunctionType.Relu
    Sqrt = mybir.ActivationFunctionType.Sqrt
    Copy = mybir.ActivationFunctionType.Copy
    KCT = KC * 128

    nc = bacc.Bacc("TRN2", target_bir_lowering=False, debug=False)

    xq = nc.dram_tensor("xq", [LQC, D], bf16, kind="ExternalInput")
    xk = nc.dram_tensor("xk", [KCT, D], bf16, kind="ExternalInput")
    xv = nc.dram_tensor("xv", [KCT, D], bf16, kind="ExternalInput")
    mbias = nc.dram_tensor("mbias", [128, KC], f32, kind="ExternalInput")
    wq = nc.dram_tensor("wq", [D, D], bf16, kind="ExternalInput")
    wk = nc.dram_tensor("wk", [D, D], bf16, kind="ExternalInput")
    wv = nc.dram_tensor("wv", [D, D], bf16, kind="ExternalInput")
    wo = nc.dram_tensor("wo", [D, D], bf16, kind="ExternalInput")
    w1 = nc.dram_tensor("w1", [D, D], bf16, kind="ExternalInput")
    w2 = nc.dram_tensor("w2", [D, D], bf16, kind="ExternalInput")
    b1c = nc.dram_tensor("b1c", [128, C], f32, kind="ExternalInput")
    b2v = nc.dram_tensor("b2v", [D], bf16, kind="ExternalInput")
    gv = nc.dram_tensor("gv", [D], f32, kind="ExternalInput")
    bv = nc.dram_tensor("bv", [D], f32, kind="ExternalInput")
    yout = nc.dram_tensor("yout", [LQC, D], f32, kind="ExternalOutput")

    def w_ap(w):
        # [128(din part), C(din chunk), D(dout)] view of a [D, D] weight
        return w.ap().rearrange("(c p) n -> p c n", p=128)

    def bcast_ap(v):
        a = v.ap()
        return bass.AP(tensor=a.tensor, offset=a.offset, ap=[[0, 128]] + list(a.ap))

    # exp groups over key chunks: pairs among fully-valid chunks, singles
    # (with mask bias) for the rest
    groups = []
    kc = 0
    while kc + 1 < KF:
        groups.append(((kc, kc + 1), False))
        kc += 2
    if kc < KF:
        groups.append(((kc,), False))
        kc += 1
    while kc < KC:
        groups.append(((kc,), True))
        kc += 1

    with tile.TileContext(nc) as tc:
        with tc.tile_pool(name="consts", bufs=1) as consts, \
             tc.tile_pool(name="persist", bufs=1) as persist, \
             tc.tile_pool(name="work", bufs=2) as work, \
             tc.tile_pool(name="pp", bufs=2, space="PSUM") as pp:

            # ---- constants
            ident = consts.tile([128, 128], bf16)
            make_identity(nc, ident)
            ones_t = consts.tile([128, 128], bf16)
            nc.gpsimd.memset(ones_t[0:1, :], 1.0)
            eps_t = consts.tile([128, 1], f32)
            nc.gpsimd.memset(eps_t, EPS)

            # ---- persistent activations (tags pair disjoint lifetimes)
            qT = persist.tile([128, C, LQC], bf16, tag="sQ")
            kT = persist.tile([128, C, KCT], bf16, tag="sK")
            vp = persist.tile([128, KC, H, DH + 1], bf16, tag="sV")
            xqT = persist.tile([128, C, LQC], bf16, tag="sA")
            xkT = persist.tile([128, C, KCT], bf16, tag="sB")
            xvT = persist.tile([128, C, KCT], bf16, tag="sC")
            o_sb = persist.tile([128, QB, D], bf16, tag="sO")
            hT = persist.tile([128, C, LQC], bf16, tag="sH")
            wq_t = persist.tile([128, C, D], bf16, tag="wA")
            wk_t = persist.tile([128, C, D], bf16, tag="wB")
            wv_t = persist.tile([128, C, D], bf16, tag="wC")

            # denominator ones-column of vp (softmax sum via the o-matmul)
            nc.vector.memset(vp[:, :, :, DH:DH + 1], 1.0)

            # ---- input loads: x^T via DMA XBAR transpose + weights in
            # consumption order.  Issues alternate across the three
            # HWDGE-capable engines (SP/ACT/DVE are all idle at start and
            # a single queue's ~1.2us per-issue rate would gate startup);
            # late-needed broadcast consts (slow gpsimd SWDGE round-trips)
            # go last.
            mb = consts.tile([128, KC], f32)
            nc.sync.dma_start(out=mb, in_=mbias.ap())
            b1_t = consts.tile([128, C], f32)
            nc.sync.dma_start(out=b1_t, in_=b1c.ap())
            b2_t = consts.tile([128, D], bf16)
            nc.sync.dma_start(out=b2_t[0:1, :], in_=b2v.ap())

            for c in range(C):
                nc.sync.dma_start_transpose(
                    xqT[:, c, :], xq.ap()[:, c * 128:(c + 1) * 128])
            nc.sync.dma_start(out=wq_t, in_=w_ap(wq))
            for c in range(C):
                nc.sync.dma_start_transpose(
                    xkT[:, c, :], xk.ap()[:, c * 128:(c + 1) * 128])
            nc.sync.dma_start(out=wk_t, in_=w_ap(wk))
            nc.sync.dma_start(out=wv_t, in_=w_ap(wv))
            for c in range(C):
                nc.sync.dma_start_transpose(
                    xvT[:, c, :], xv.ap()[:, c * 128:(c + 1) * 128])

            # FFN weights prefetched into their own slots
            wo_t = persist.tile([128, C, D], bf16, tag="wD")
            w1_t = persist.tile([128, C, D], bf16, tag="wE")
            w2_t = persist.tile([128, C, D], bf16, tag="wF")
            nc.sync.dma_start(out=wo_t, in_=w_ap(wo))
            nc.sync.dma_start(out=w1_t, in_=w_ap(w1))
            nc.sync.dma_start(out=w2_t, in_=w_ap(w2))
            g_t = consts.tile([128, D], f32)
            nc.gpsimd.dma_start(out=g_t, in_=bcast_ap(gv))
            be_t = consts.tile([128, D], f32)
            nc.gpsimd.dma_start(out=be_t, in_=bcast_ap(bv))
            # oT reuses xkT's slot (xqT stays live: its qt=1 projection
            # chains run as phase-2 fillers after oT writes begin)
            oT = persist.tile([128, C, LQC], bf16, tag="sB")
            attT = persist.tile([128, C, LQC], bf16, tag="sC")

            # eviction engine rotation (keeps DVE from being the choke point)
            ev_state = [0]

            def evict(out_ap, in_ap, engines="sv"):
                e = engines[ev_state[0] % len(engines)]
                ev_state[0] += 1
                if e == "s":
                    nc.scalar.activation(out=out_ap, in_=in_ap, func=Copy,
                                         bias=0.0, scale=1.0)
                else:
                    nc.vector.tensor_copy(out=out_ap, in_=in_ap)

            # ---------------- filler chains ----------------
            def qk_chain(w_t, x_t, out_t, n, q0, qw, engines):
                def fn():
                    ps = pp.tile([128, 512], f32, tag="p512", name=f"ps_{n}_{q0}")
                    for c in range(C):
                        nc.tensor.matmul(ps[:, 0:qw],
                                         w_t[:, c, n * 128:(n + 1) * 128],
                                         x_t[:, c, q0:q0 + qw],
                                         start=(c == 0), stop=(c == C - 1))
                    evict(out_t[:, n, q0:q0 + qw], ps[:, 0:qw], engines)
                return fn

            def v_chain(hp, kt):
                def fn():
                    ps = pp.tile([128, 512], f32, tag="p512", name=f"psv_{hp}_{kt}")
                    for c in range(C):
                        nc.tensor.matmul(ps[:, 0:128],
                                         xvT[:, c, kt * 128:(kt + 1) * 128],
                                         wv_t[:, c, hp * 128:(hp + 1) * 128],
                                         start=(c == 0), stop=(c == C - 1))
                    evict(vp[:, kt, 2 * hp:2 * hp + 2, 0:DH],
                          ps[:, 0:128].rearrange("p (h d) -> p h d", d=DH), "v")
                return fn

            def oT_xbar(c, qbs):
                # 128x128 bf16 SBUF->SBUF transposes on the DMA XBAR
                def fn():
                    for qb in qbs:
                        nc.sync.dma_start_transpose(
                            oT[:, c, qb * 128:(qb + 1) * 128],
                            o_sb[:, qb, c * 128:(c + 1) * 128])
                return fn

            def attT_chain(n, qc):
                # attT[n-chunk, q] = Wo[:, n-chunk]^T @ oT[:, q]
                def fn():
                    ps = pp.tile([128, 512], f32, tag="p512", name=f"psa_{n}_{qc}")
                    for c in range(C):
                        nc.tensor.matmul(ps[:],
                                         wo_t[:, c, n * 128:(n + 1) * 128],
                                         oT[:, c, qc * 512:(qc + 1) * 512],
                                         start=(c == 0), stop=(c == C - 1))
                    evict(attT[:, n, qc * 512:(qc + 1) * 512], ps[:], "v")
                return fn

            def w1_chain(n, qc, engines):
                def fn():
                    ps = pp.tile([128, 512], f32, tag="p512", name=f"ps1_{n}_{qc}")
                    for c in range(C):
                        nc.tensor.matmul(ps[:],
                                         w1_t[:, c, n * 128:(n + 1) * 128],
                                         attT[:, c, qc * 512:(qc + 1) * 512],
                                         start=(c == 0), stop=(c == C - 1))
                    e = engines[ev_state[0] % len(engines)]
                    ev_state[0] += 1
                    if e == "s":
                        nc.scalar.activation(
                            out=hT[:, n, qc * 512:(qc + 1) * 512], in_=ps[:],
                            func=Relu, bias=b1_t[:, n:n + 1], scale=1.0)
                    else:
                        nc.vector.tensor_scalar(
                            out=hT[:, n, qc * 512:(qc + 1) * 512], in0=ps[:],
                            scalar1=b1_t[:, n:n + 1], scalar2=0.0,
                            op0=mybir.AluOpType.add, op1=mybir.AluOpType.max)
                return fn

            # per-qb layernorm state, consumed by the tail region
            ysb_t = [None] * QB
            mv_t = [None] * QB

            def w2_stats_chain(qb):
                # y = hT^T @ W2 + attT^T (residual) + ones x b2, then
                # evict + bn stats; the sqrt-dependent part runs in the tail
                def fn():
                    ps = pp.tile([128, 2, 512], f32, tag="pair", name=f"psy_{qb}")
                    for n0, nw, i in ((0, 512, 0), (512, 256, 1)):
                        for c in range(C):
                            nc.tensor.matmul(ps[:, i, 0:nw],
                                             hT[:, c, qb * 128:(qb + 1) * 128],
                                             w2_t[:, c, n0:n0 + nw],
                                             start=(c == 0), stop=False)
                        # residual: += attT^T via identity rhs, per n-chunk
                        for cn in range(n0 // 128, (n0 + nw) // 128):
                            nc.tensor.matmul(
                                ps[:, i, cn * 128 - n0:(cn + 1) * 128 - n0],
                                attT[:, cn, qb * 128:(qb + 1) * 128],
                                ident[:],
                                start=False, stop=False, skip_group_check=True)
                        # += b2 via rank-1 ones matmul
                        nc.tensor.matmul(ps[:, i, 0:nw], ones_t[0:1, :],
                                         b2_t[0:1, n0:n0 + nw],
                                         start=False, stop=True,
                                         skip_group_check=True)
                    ysb = work.tile([128, D], bf16, tag="ysb", bufs=QB,
                                    name=f"y_{qb}")
                    nc.vector.tensor_copy(out=ysb[:, 0:512], in_=ps[:, 0, :])
                    nc.vector.tensor_copy(out=ysb[:, 512:768],
                                          in_=ps[:, 1, 0:256])
                    stats = work.tile([128, 3, 6], f32, tag="stats",
                                      name=f"st_{qb}")
                    for sg in range(3):
                        nc.vector.bn_stats(out=stats[:, sg, :],
                                           in_=ysb[:, sg * 256:(sg + 1) * 256])
                    mv = work.tile([128, 2], f32, tag="mv", bufs=QB,
                                   name=f"mv_{qb}")
                    nc.vector.bn_aggr(out=mv[:], in_=stats[:])
                    ysb_t[qb] = ysb
                    mv_t[qb] = mv
                return fn

            Identity = mybir.ActivationFunctionType.Identity

            def ln_tail(qb):
                ysb, mv = ysb_t[qb], mv_t[qb]
                rstd = work.tile([128, 1], f32, tag="rstd", bufs=3,
                                 name=f"rs_{qb}")
                nc.scalar.activation(out=rstd[:], in_=mv[:, 1:2], func=Sqrt,
                                     bias=eps_t[:], scale=1.0)
                nc.vector.reciprocal(rstd[:], rstd[:])
                # (y - mu)*rstd on ACT: Identity(y*rstd + (-mu*rstd))
                nmu = work.tile([128, 1], f32, tag="nmu", bufs=3,
                                name=f"nm_{qb}")
                nc.vector.tensor_scalar(
                    out=nmu[:], in0=mv[:, 0:1], scalar1=rstd[:], scalar2=-1.0,
                    op0=mybir.AluOpType.mult, op1=mybir.AluOpType.mult)
                yn = work.tile([128, D], f32, tag="yn", bufs=3, name=f"yn_{qb}")
                nc.scalar.activation(out=yn[:], in_=ysb[:], func=Identity,
                                     bias=nmu[:], scale=rstd[:])
                if qb % 2 == 0:
                    nc.vector.tensor_mul(out=yn[:], in0=yn[:], in1=g_t[:])
                    nc.gpsimd.tensor_add(out=yn[:], in0=yn[:], in1=be_t[:])
                else:
                    nc.gpsimd.tensor_mul(out=yn[:], in0=yn[:], in1=g_t[:])
                    nc.vector.tensor_add(out=yn[:], in0=yn[:], in1=be_t[:])
                nc.scalar.dma_start(out=yout.ap()[qb * 128:(qb + 1) * 128, :],
                                    in_=yn[:])

            # ---------------- filler queue ----------------
            fillers = []   # list of (round, fn)
            fpos = [0]

            def drain(k):
                n = 0
                while fpos[0] < len(fillers) and n < k:
                    fillers[fpos[0]][1]()
                    fpos[0] += 1
                    n += 1

            def force_round(r):
                while fpos[0] < len(fillers) and fillers[fpos[0]][0] <= r:
                    fillers[fpos[0]][1]()
                    fpos[0] += 1

            # phase-1 fillers: qt=0 projections only (qt=1 chains feed the
            # qc=1 attention and become phase-2 fillers for balance)
            kw = [(k0, min(512, KCT - k0)) for k0 in range(0, KCT, 512)]
            for n in range(C):
                fillers.append((0, qk_chain(wq_t, xqT, qT, n, 0, 512, "v")))
            for n in range(C):
                for (k0, wdt) in kw:
                    fillers.append((n, qk_chain(wk_t, xkT, kT, n, k0, wdt,
                                                "v")))
                for kt in range(KC):
                    fillers.append((n, v_chain(n, kt)))

            # rounds 0-1 upfront (attention head 0 needs round 0; round 1
            # keeps PE fed while the head-0 DMAs land)
            force_round(1)

            # ---------------- attention ----------------
            # software-pipelined: scores for step idx+1 are emitted before
            # the o-matmuls of step idx, so the PE never sits behind the
            # exp latency
            NG = len(groups)

            def attn_phase(qc, post_head=None):
                seq = [(h, gi) for h in range(H) for gi in range(NG)]
                pos_t = {}
                pair_t = {}

                def emit_scores(idx):
                    h, gi = seq[idx]
                    p0 = (h % 2) * 64
                    cc = h // 2
                    if gi == 0:
                        force_round(min(cc, C - 1))
                        pos = pp.tile([128, 4, DH + 1], f32, tag="pos",
                                      name=f"pos_{qc}_{h}")
                        # 4 accumulation groups share this bank: a
                        # start=True reset wipes the whole bank, so zero it
                        # once and accumulate only
                        nc.vector.memset(pos[:], 0.0)
                        pos_t[h] = pos
                    g, _ = groups[gi]
                    ps_s = pp.tile([128, 2, 512], f32, tag="pair",
                                   name=f"pss_{qc}_{h}_{gi}")
                    for i, kc in enumerate(g):
                        nc.tensor.matmul(
                            ps_s[:, i, :],
                            kT[p0:p0 + 64, cc, kc * 128:(kc + 1) * 128],
                            qT[p0:p0 + 64, cc, qc * 512:(qc + 1) * 512],
                            start=True, stop=True)
                    pair_t[(h, gi)] = ps_s
                    if gi == 0:
                        force_round(min(cc + 2, C - 1))

                emit_scores(0)
                for idx, (h, gi) in enumerate(seq):
                    g, biased = groups[gi]
                    ps_s = pair_t.pop((h, gi))
                    ex = work.tile([128, 2, 512], bf16, tag="ex", bufs=3,
                                   name=f"ex_{qc}_{h}_{gi}")
                    glen = len(g)
                    bias = mb[:, g[0]:g[0] + 1] if biased else 0.0
                    nc.scalar.activation(out=ex[:, 0:glen, :],
                                         in_=ps_s[:, 0:glen, :], func=Exp,
                                         bias=bias, scale=1.0)
                    if idx + 1 < len(seq):
                        emit_scores(idx + 1)
                    drain(1)
                    pos = pos_t[h]
                    for i, kc in enumerate(g):
                        for qs in range(4):
                            nc.tensor.matmul(
                                pos[:, qs, :],
                                ex[:, i, qs * 128:(qs + 1) * 128],
                                vp[:, kc, h, :],
                                start=False, stop=(kc == KC - 1),
                                skip_group_check=True)
                    if gi == NG - 1:
                        # normalize -> o_sb
                        pos = pos_t.pop(h)
                        rec = work.tile([128, 4, 1], f32, tag="rec", bufs=3,
                                        name=f"rec_{qc}_{h}")
                        nc.vector.reciprocal(rec[:], pos[:, :, DH:DH + 1])
                        for qs in range(4):
                            qb = qc * 4 + qs
                            nc.vector.tensor_scalar_mul(
                                out=o_sb[:, qb, h * DH:(h + 1) * DH],
                                in0=pos[:, qs, 0:DH],
                                scalar1=rec[:, qs, 0:1])
                        if post_head is not None:
                            post_head(h)

            # qc=0: fillers are the projection chains
            attn_phase(0)
            force_round(C)

            # qc=1: fillers are the qt=1 projections + the qc=0 FFN pipeline
            fillers.clear()
            fpos[0] = 0
            for c in range(C):
                oT_xbar(c, (0, 1, 2, 3))()
            for n in range(C):
                fillers.append((n, qk_chain(wq_t, xqT, qT, n, 512, 512, "v")))
                fillers.append((n, attT_chain(n, 0)))
            for n in range(C):
                # w1 contracts over ALL attT chunks -> must follow attT(5)
                fillers.append((C - 1, w1_chain(n, 0, "v")))
            for qb in range(4):
                fillers.append((C, w2_stats_chain(qb)))

            def post_head_qc1(h):
                if h % 2 == 1:
                    # o_sb columns for chunk h//2 are complete for qc=1
                    oT_xbar(h // 2, (4, 5, 6, 7))()

            attn_phase(1, post_head=post_head_qc1)
            drain(len(fillers))

            # ---------------- qc=1 FFN tail ----------------
            for n in range(C):
                attT_chain(n, 1)()
            for n in range(C):
                w1_chain(n, 1, "sv")()
            for qb in range(4, QB):
                w2_stats_chain(qb)()
            # layernorm tail: all Sqrt usage lives here, so the ACT table
            # set is swapped exactly once for the whole kernel
            for qb in range(QB):
                ln_tail(qb)

    nc.compile()
    return nc


def _get_nc():
    # returns the most recently used compiled module (for test harness)
    key = _CACHE.get("last_key")
    if key is None:
        # default shape for this problem's input (valid_lens ~1028/1044/996)
        key = (9, 7)
    if ("nc", key) not in _CACHE:
        _CACHE[("nc", key)] = _build(*key)
    _CACHE["last_key"] = key
    return _CACHE[("nc", key)]


def _prepare_in_maps(queries, keys, values, mask, Wq, Wk, Wv, Wo, W1, b1,
                     W2, b2, ln_g, ln_b):
    bf16 = ml_dtypes.bfloat16
    queries = np.asarray(queries, dtype=np.float32)
    keys = np.asarray(keys, dtype=np.float32)
    values = np.asarray(values, dtype=np.float32)
    mask = np.asarray(mask)

    valid = (mask != 0).sum(axis=1).astype(np.int64)        # [B]
    valid = np.maximum(valid, 1)
    KC = int(-(-valid.max() // 128))
    KF = int(valid.min() // 128)
    KCT = KC * 128

    kidx = np.arange(KCT)
    mb_all = np.where(kidx[None, :] < valid[:, None], 0.0, -1e6)
    mb_all = mb_all.astype(np.float32).reshape(B, KC, 128).transpose(0, 2, 1)

    wq_s = (np.asarray(Wq, np.float32) / np.sqrt(np.float32(DH)))
    common = {
        "wq": wq_s.astype(bf16),
        "wk": np.asarray(Wk, np.float32).astype(bf16),
        "wv": np.asarray(Wv, np.float32).astype(bf16),
        "wo": np.asarray(Wo, np.float32).astype(bf16),
        "w1": np.asarray(W1, np.float32).astype(bf16),
        "w2": np.asarray(W2, np.float32).astype(bf16),
        "b1c": np.ascontiguousarray(
            np.asarray(b1, np.float32).reshape(C, 128).T),
        "b2v": np.asarray(b2, np.float32).astype(bf16),
        "gv": np.ascontiguousarray(ln_g, np.float32),
        "bv": np.ascontiguousarray(ln_b, np.float32),
    }

    in_maps = []
    for core in range(NC):
        b, half = core // 2, core % 2
        in_maps.append(dict(
            common,
            xq=np.ascontiguousarray(
                queries[b, half * LQC:(half + 1) * LQC, :]).astype(bf16),
            xk=np.ascontiguousarray(keys[b, :KCT, :]).astype(bf16),
            xv=np.ascontiguousarray(values[b, :KCT, :]).astype(bf16),
            mbias=np.ascontiguousarray(mb_all[b]),
        ))
    return in_maps, (KC, KF)


def kernel(queries, keys, values, mask, Wq, Wk, Wv, Wo, W1, b1, W2, b2,
           ln_g, ln_b, _trace=False):
    from concourse.bass_utils import run_bass_kernel_spmd

    in_maps, key = _prepare_in_maps(queries, keys, values, mask, Wq, Wk, Wv,
                                    Wo, W1, b1, W2, b2, ln_g, ln_b)
    if ("nc", key) not in _CACHE:
        _CACHE[("nc", key)] = _build(*key)
    _CACHE["last_key"] = key
    nc = _CACHE[("nc", key)]
    res = run_bass_kernel_spmd(nc, in_maps, core_ids=list(range(NC)),
                               trace=_trace)
    _CACHE["last_result"] = res

    out = np.empty((B, LQ, D), dtype=np.float32)
    for core in range(NC):
        b, half = core // 2, core % 2
        out[b, half * LQC:(half + 1) * LQC, :] = res.results[core]["yout"]
    return out
